# revision 25
# baseline (speedup 1.0000x reference)
"""Trainium2 Bass kernel for a Qwen2-VL vision transformer block.

Strategy: 8-way sequence-parallel across NeuronCores. Each core owns a
256-row shard of the 2048-token sequence and the full weights (bf16).
K/V for the full sequence are exchanged with two AllGathers (V first,
then K) so the gathers overlap the Q-side projections; every other
stage is perfectly partitioned. All matmuls run on the PE in bf16 with
fp32 PSUM accumulation; LayerNorm statistics, the softmax normalizer,
and residual adds stay in fp32 (RoPE arithmetic is bf16).

Layout notes:
  - Projections produce activations in natural [seq, feat] layout; the
    PE transpose (identity matmul) produces the [feat, seq] operands
    that later matmuls need as stationary input.
  - Attention computes scores^T [key, query] per head so the softmax
    denominator can be accumulated by an extra all-ones column on the
    stationary V operand, and the per-query normalizer is broadcast
    across partitions with a rank-1 outer-product matmul.
  - LayerNorm affine params and the quick-gelu 1.702 scale are folded
    into the weights on the host, which is exact in fp32.
  - W1 is mostly prefetched into SBUF during the attention phase so the
    MLP runs PE-bound instead of DMA-bound.
"""

import os
import sys

import numpy as np

for _p in ("/opt/trn_rl_repo",):
    if _p not in sys.path:
        sys.path.insert(0, _p)

import ml_dtypes  # noqa: E402


BF = ml_dtypes.bfloat16

B, S, H = 1, 2048, 1280
NH, HD = 16, 80
MLP = 5120
EPS = 1e-6
NCORES = 8
SL = S // NCORES            # 256 sequence rows per core
SB = SL // 128              # 2 partition blocks per core
HC = H // 128               # 10 contraction chunks over H
MC = MLP // 128             # 40 blocks over the MLP dim
KB = S // 128               # 16 key blocks over the full sequence
NCOLS = ((0, 512), (512, 512), (1024, 256))
SCALE = 1.0 / float(np.sqrt(np.float32(HD)))
KT_ELEMS = NH * HD * SL     # 327680, gathered K^T region
VROW = NH * (HD + 1)        # 1296, V row with interleaved ones column
V_ELEMS = SL * VROW         # gathered V region (ones travel with it)
KVE = KT_ELEMS + V_ELEMS
NCOLSV = ((0, 320), (320, 320), (640, 320), (960, 320))
W1PRE = 8                   # W1 blocks prefetched into SBUF before MLP


def _build_bass(use_bias):
    import bass_rust
    import concourse.bacc as bacc
    import concourse.tile as tile
    from concourse import mybir
    from concourse.masks import make_identity

    F32 = mybir.dt.float32
    BF16 = mybir.dt.bfloat16
    FP8 = mybir.dt.float8e4
    AF = mybir.ActivationFunctionType
    OP = mybir.AluOpType

    nc = bacc.Bacc("TRN2", target_bir_lowering=False, debug=False,
                   num_devices=NCORES)

    x_io = nc.dram_tensor("x_loc", [SL, H], F32, kind="ExternalInput")
    xt_io = nc.dram_tensor("xT", [H, SL], FP8, kind="ExternalInput")
    wsum_io = nc.dram_tensor("wsum3", [1, 3 * H], BF16, kind="ExternalInput")
    cos_io = nc.dram_tensor("cosr", [SL, H], BF16, kind="ExternalInput")
    sin_io = nc.dram_tensor("sins", [SL, H], BF16, kind="ExternalInput")
    wqt_io = nc.dram_tensor("wqt", [H, H], FP8, kind="ExternalInput")
    wkt_io = nc.dram_tensor("wkt", [H, H], FP8, kind="ExternalInput")
    wvt_io = nc.dram_tensor("wvt", [H, H], FP8, kind="ExternalInput")
    wot_io = nc.dram_tensor("wot", [H, H], BF16, kind="ExternalInput")
    w1b_io = nc.dram_tensor("w1b", [MC, 128, HC, 128], BF16,
                            kind="ExternalInput")
    w2t_io = nc.dram_tensor("w2t", [MLP, H], BF16, kind="ExternalInput")
    bias5_io = nc.dram_tensor("bias5", [5, H], BF16, kind="ExternalInput")
    b1s_io = nc.dram_tensor("b1s", [128, MC], F32, kind="ExternalInput")
    out_io = nc.dram_tensor("out_loc", [SL, H], F32, kind="ExternalOutput")

    cc_in = nc.dram_tensor("cc_in", [KVE], FP8)
    cc_out = nc.dram_tensor("cc_out", [NCORES, KVE], FP8,
                            addr_space="Shared")

    with tile.TileContext(nc) as tc:
        _qrr = [nc.sync, nc.gpsimd, nc.scalar]
        _qi = [0]

        def dmaq():
            e = _qrr[_qi[0] % len(_qrr)]
            _qi[0] += 1
            return e

        const = tc.alloc_tile_pool(name="const", bufs=1)
        persist = tc.alloc_tile_pool(name="persist", bufs=1)
        misc = tc.alloc_tile_pool(name="misc", bufs=2)

        ident = const.tile([128, 128], BF16, name="ident", tag="ident")
        make_identity(nc, ident)
        ones_b = const.tile([1, 128], BF16, name="ones_b", tag="ones_b")
        nc.vector.memset(ones_b, 1.0)
        ones_f = const.tile([1, 128], F32, name="ones_f", tag="ones_f")
        nc.vector.memset(ones_f, 1.0)
        eps_t = const.tile([128, 1], F32, name="eps_t", tag="eps_t")
        nc.vector.memset(eps_t, EPS)
        bias_t = []
        if use_bias:
            for bi in range(5):
                bt = const.tile([1, H], BF16, name=f"bias{bi}",
                                tag=f"bias{bi}")
                nc.sync.dma_start(out=bt, in_=bias5_io[bi:bi + 1, :])
                bias_t.append(bt)
        x_sb = [persist.tile([128, H], F32, name=f"x{sb}", tag=f"x{sb}")
                for sb in range(SB)]
        for sb in range(SB):
            nc.sync.dma_start(out=x_sb[sb], in_=x_io[sb * 128:(sb + 1) * 128, :])
        wsum_t = const.tile([1, 3 * H], BF16, name="wsum", tag="wsum")
        nc.scalar.dma_start(out=wsum_t, in_=wsum_io[:, :])
        b1s = const.tile([128, MC], F32, name="b1s", tag="b1s")
        nc.gpsimd.dma_start(out=b1s, in_=b1s_io[:, :])
        x2_sb = [persist.tile([128, H], F32, name=f"x2_{sb}", tag=f"x2_{sb}")
                 for sb in range(SB)]
        qt = [persist.tile([HD, SL], FP8, name=f"qt{h}", tag=f"qt{h}")
              for h in range(NH)]
        attnT = [persist.tile([HD, SL], BF16, name=f"attnT{h}", tag=f"attnT{h}")
                 for h in range(NH)]

        def layernorm_bf16(src, dst):
            # dst[sb] = (src[sb] - mean) * rsqrt(var + eps), cast to bf16
            for sb in range(SB):
                stats = misc.tile([128, 5, 6], F32, name=f"lnst{sb}", tag="lnst")
                sv = src[sb].rearrange("p (g d) -> p g d", d=256)
                for g in range(5):
                    nc.vector.bn_stats(out=stats[:, g, :], in_=sv[:, g, :])
                mv = misc.tile([128, 2], F32, name=f"lnmv{sb}", tag="lnmv")
                nc.vector.bn_aggr(out=mv, in_=stats)
                rstd = misc.tile([128, 1], F32, name=f"lnrs{sb}", tag="lnrs")
                nc.scalar.activation(out=rstd, in_=mv[:, 1:2], func=AF.Sqrt,
                                     bias=eps_t)
                nc.vector.reciprocal(out=rstd, in_=rstd)
                nc.vector.tensor_scalar(out=dst[sb], in0=src[sb],
                                        scalar1=mv[:, 0:1], scalar2=rstd,
                                        op0=OP.subtract, op1=OP.mult)

        def transpose_to(src, dst, ps_pool):
            # src: SB tiles [128, H] bf16 -> dst: HC tiles [128, SL] bf16
            insts = []
            for hc in range(HC):
                for sb in range(SB):
                    pt = ps_pool.tile([128, 128], BF16, name="pt", tag="pt")
                    tr = nc.tensor.transpose(pt,
                                             src[sb][:, hc * 128:(hc + 1) * 128],
                                             ident)
                    insts.append(tr)
                    nc.vector.tensor_copy(
                        out=dst[hc][:, sb * 128:(sb + 1) * 128], in_=pt)
            return insts

        # ================= phase A: LN1, QKV, RoPE, head transposes ====
        p_ln = tc.alloc_tile_pool(name="p_ln", bufs=1)
        p_qkv = tc.alloc_tile_pool(name="p_qkv", bufs=1)
        wpool = tc.alloc_tile_pool(name="wpool", bufs=2)
        psA_tr = tc.alloc_tile_pool(name="psA_tr", bufs=2, space="PSUM")
        psA_mm = tc.alloc_tile_pool(name="psA_mm", bufs=2, space="PSUM")
        ps_warm = tc.alloc_tile_pool(name="ps_warm", bufs=2, space="PSUM")

        # Keep the PE busy (HAM warm) while the first DMAs and LayerNorm
        # run: a burst of small identity matmuls chained into one
        # accumulator so the scheduler cannot float them into later
        # phases (their PSUM writes would clobber live accumulators).
        WARMUP = True
        if WARMUP:
            wps = ps_warm.tile([128, 128], F32, name="warm", tag="warm")
            warm_insts = []
            for wi in range(8):
                mm = nc.tensor.matmul(wps, lhsT=ident, rhs=ident,
                                      start=(wi == 0), stop=(wi == 7))
                warm_insts.append(mm)

        # x^T comes precomputed from the host (bf16); LayerNorm is applied
        # algebraically: q = r * (x @ W - mu * colsum(W)) via one extra
        # rank-1 matmul row per accumulation group and a per-token scale
        # folded into the PSUM-evacuation copy.
        xTall = p_ln.tile([128, HC, SL], FP8, name="xTall", tag="xTall")
        nc.sync.dma_start(out=xTall,
                          in_=xt_io.rearrange("(hc p) s -> p hc s", p=128))
        mun = []
        rstd_sb = []
        for sb in range(SB):
            stats = misc.tile([128, 5, 6], F32, name=f"lnst{sb}", tag="lnst")
            sv = x_sb[sb].rearrange("p (g d) -> p g d", d=256)
            for g in range(5):
                nc.vector.bn_stats(out=stats[:, g, :], in_=sv[:, g, :])
            mv = misc.tile([128, 2], F32, name=f"lnmv{sb}", tag="lnmv")
            nc.vector.bn_aggr(out=mv, in_=stats)
            rstd = p_ln.tile([128, 1], F32, name=f"rstd{sb}", tag=f"rstd{sb}")
            nc.scalar.activation(out=rstd, in_=mv[:, 1:2], func=AF.Sqrt,
                                 bias=eps_t)
            nc.vector.reciprocal(out=rstd, in_=rstd)
            nc.vector.tensor_scalar(out=rstd, in0=rstd, scalar1=1.0 / 64.0,
                                    scalar2=None, op0=OP.mult)
            rstd_sb.append(rstd)
            mneg = p_ln.tile([128, 1], BF16, name=f"mneg{sb}", tag=f"mneg{sb}")
            nc.vector.tensor_scalar(out=mneg, in0=mv[:, 0:1], scalar1=-1.0,
                                    scalar2=None, op0=OP.mult)
            mrow = p_ln.tile([1, 128], BF16, name=f"mrow{sb}", tag=f"mrow{sb}")
            mps = psA_tr.tile([1, 128], BF16, name=f"mps{sb}", tag="mps")
            nc.tensor.transpose(mps, mneg, ident)
            nc.vector.tensor_copy(out=mrow, in_=mps)
            mun.append(mrow)

        def project(w_io, bias_idx, store, cols=NCOLS, wq_eng=None):
            wt = []
            for hc in range(HC):
                w = wpool.tile([128, H], FP8, name=f"w{hc}", tag=f"w{hc}")
                (wq_eng or dmaq()).dma_start(
                    out=w, in_=w_io[hc * 128:(hc + 1) * 128, :])
                wt.append(w)
            for sb in range(SB):
                for (c0, cn) in cols:
                    ps = psA_mm.tile([128, 512], F32, name="mmps", tag="mmps")
                    for hc in range(HC):
                        nc.tensor.matmul(
                            ps[:, 0:cn],
                            lhsT=xTall[:, hc, sb * 128:(sb + 1) * 128],
                            rhs=wt[hc][:, c0:c0 + cn],
                            start=(hc == 0), stop=False)
                    nc.tensor.matmul(
                        ps[:, 0:cn], lhsT=mun[sb],
                        rhs=wsum_t[:, bias_idx * H + c0:bias_idx * H + c0 + cn],
                        start=False, stop=(not use_bias))
                    if use_bias:
                        nc.tensor.matmul(
                            ps[:, 0:cn], lhsT=ones_b,
                            rhs=bias_t[bias_idx][:, c0:c0 + cn],
                            start=False, stop=True)
                    store(sb, c0, cn, ps)

        # ---- K & V projections; publish both, then one AllGather ----
        # V is stored pre-interleaved with its softmax ones-column so the
        # gathered V can be consumed with contiguous DMAs and no memsets.
        vloc = p_qkv.tile([128, SB, VROW], FP8, name="vloc", tag="vloc")
        v4 = vloc.rearrange("p b (h c) -> p b h c", c=HD + 1)
        nc.vector.memset(v4[:, :, :, HD:HD + 1], 1.0)

        def store_v(sb, c0, cn, ps):
            g0 = c0 // HD
            gn = cn // HD
            nc.scalar.activation(
                out=v4[:, sb, g0:g0 + gn, 0:HD],
                in_=ps[:, 0:cn].rearrange("p (h c) -> p h c", c=HD),
                func=AF.Copy, scale=rstd_sb[sb])

        cosr = [p_qkv.tile([128, H], BF16, name=f"cos{sb}", tag=f"cos{sb}")
                for sb in range(SB)]
        sins = [p_qkv.tile([128, H], BF16, name=f"sin{sb}", tag=f"sin{sb}")
                for sb in range(SB)]
        for sb in range(SB):
            nc.scalar.dma_start(out=cosr[sb],
                                in_=cos_io[sb * 128:(sb + 1) * 128, :])
            nc.scalar.dma_start(out=sins[sb],
                                in_=sin_io[sb * 128:(sb + 1) * 128, :])

        def rope(nat, out):
            for sb in range(SB):
                tmp = misc.tile([128, H], BF16, name="ropetmp", tag="ropetmp")
                t3 = tmp.rearrange("p (h c) -> p h c", c=HD)
                q3 = nat[sb].rearrange("p (h c) -> p h c", c=HD)
                s3 = sins[sb].rearrange("p (h c) -> p h c", c=HD)
                nc.vector.tensor_mul(out=t3[:, :, 0:40], in0=q3[:, :, 40:80],
                                     in1=s3[:, :, 0:40])
                nc.vector.tensor_mul(out=t3[:, :, 40:80], in0=q3[:, :, 0:40],
                                     in1=s3[:, :, 40:80])
                nc.vector.tensor_mul(out=nat[sb], in0=nat[sb], in1=cosr[sb])
                nc.vector.tensor_add(out=out[sb], in0=nat[sb], in1=tmp)

        knat = [p_qkv.tile([128, H], BF16, name=f"kn{sb}", tag=f"kn{sb}")
                for sb in range(SB)]
        krope = [p_qkv.tile([128, H], BF16, name=f"kr{sb}", tag=f"kr{sb}")
                 for sb in range(SB)]
        project(wkt_io, 1,
                lambda sb, c0, cn, ps: nc.scalar.activation(
                    out=knat[sb][:, c0:c0 + cn], in_=ps[:, 0:cn],
                    func=AF.Copy, scale=rstd_sb[sb]),
                wq_eng=nc.gpsimd)
        rope(knat, krope)
        project(wvt_io, 2, store_v, cols=NCOLSV, wq_eng=nc.gpsimd)
        nc.sync.dma_start(
            out=cc_in[KT_ELEMS:KVE].rearrange("(b p f) -> p b f",
                                              p=128, f=VROW),
            in_=vloc)
        ktall = p_qkv.tile([HD, NH, SL], FP8, name="ktall", tag="ktall")
        for h in range(NH):
            for sb in range(SB):
                ptk = psA_tr.tile([HD, 128], BF16, name="ptk", tag="pt")
                nc.tensor.transpose(ptk, krope[sb][:, h * HD:(h + 1) * HD], ident)
                nc.vector.tensor_copy(
                    out=ktall[:, h, sb * 128:(sb + 1) * 128], in_=ptk)
        nc.sync.dma_start(
            out=cc_in[0:KT_ELEMS].rearrange("(d h s) -> d h s", h=NH, s=SL),
            in_=ktall)
        bar_cc = nc.gpsimd.collective_compute(
            "AllGather", OP.bypass,
            replica_groups=[list(range(NCORES))],
            ins=[cc_in.ap()], outs=[cc_out.ap()])

        # ---- Q: project, RoPE, per-head transpose (overlaps gathers) ----
        qnat = [p_qkv.tile([128, H], BF16, name=f"qn{sb}", tag=f"qn{sb}")
                for sb in range(SB)]
        qrope = [p_qkv.tile([128, H], BF16, name=f"qr{sb}", tag=f"qr{sb}")
                 for sb in range(SB)]
        project(wqt_io, 0,
                lambda sb, c0, cn, ps: nc.scalar.activation(
                    out=qnat[sb][:, c0:c0 + cn], in_=ps[:, 0:cn],
                    func=AF.Copy, scale=rstd_sb[sb]))
        rope(qnat, qrope)
        for h in range(NH):
            for sb in range(SB):
                ptq = psA_tr.tile([HD, 128], BF16, name="ptq", tag="pt")
                nc.tensor.transpose(ptq, qrope[sb][:, h * HD:(h + 1) * HD], ident)
                nc.vector.tensor_copy(out=qt[h][:, sb * 128:(sb + 1) * 128],
                                      in_=ptq)

        ps_warm.release()
        psA_mm.release()
        psA_tr.release()
        wpool.release()
        p_qkv.release()
        p_ln.release()

        # ================= phase B: attention ==========================
        # W1 prefetch streams on the gpsimd queue while attention runs.
        w1pre = tc.alloc_tile_pool(name="w1pre", bufs=1)
        w1pre_t = []
        w1q = [nc.sync, nc.scalar]
        for mb in range(W1PRE):
            w1 = w1pre.tile([128, HC, 128], BF16, name=f"w1p{mb}",
                            tag=f"w1p{mb}")
            w1q[mb % 2].dma_start(out=w1, in_=w1b_io[mb])
            w1pre_t.append(w1)

        p_ln2 = tc.alloc_tile_pool(name="p_ln2", bufs=1)
        wop = tc.alloc_tile_pool(name="wop", bufs=1)
        wo_t = []
        for h in range(NH):
            w = wop.tile([HD, H], BF16, name=f"wo{h}", tag=f"wo{h}")
            dmaq().dma_start(out=w, in_=wot_io[h * HD:(h + 1) * HD, :])
            wo_t.append(w)

        p_att = tc.alloc_tile_pool(name="p_att", bufs=1)
        katt = tc.alloc_tile_pool(name="katt", bufs=1)
        eatt = tc.alloc_tile_pool(name="eatt", bufs=2)
        ps_sc = tc.alloc_tile_pool(name="ps_sc", bufs=2, space="PSUM")
        ps_at = tc.alloc_tile_pool(name="ps_at", bufs=3, space="PSUM")
        ps_rb = tc.alloc_tile_pool(name="ps_rb", bufs=1, space="PSUM")

        ktr = []
        ktq = [nc.sync, nc.scalar]
        for r in range(NCORES):
            krt = katt.tile([HD, NH, SL], FP8, name=f"ktr{r}", tag=f"ktr{r}")
            src_r = cc_out[r, 0:KT_ELEMS].rearrange("(d h s) -> d h s",
                                                    h=NH, s=SL)
            kdma = ktq[r % 2].dma_start(out=krt, in_=src_r)
            bass_rust.add_dep_helper(kdma.ins, bar_cc.ins,
                                     reason="wait for remote K via barrier")
            ktr.append(krt)

        vaug = []
        for kb in range(KB):
            va = p_att.tile([128, NH, HD + 1], FP8, name=f"vaug{kb}",
                            tag=f"vaug{kb}")
            r, lb = divmod(kb, 2)
            vsrc = cc_out[r, KT_ELEMS + lb * 128 * VROW:
                          KT_ELEMS + (lb + 1) * 128 * VROW]
            vsrc = vsrc.rearrange("(p f) -> p f", f=VROW)
            vdma = nc.scalar.dma_start(
                out=va.rearrange("p a b -> p (a b)"), in_=vsrc)
            bass_rust.add_dep_helper(vdma.ins, bar_cc.ins,
                                     reason="wait for remote V via barrier")
            vaug.append(va)

        for h in range(NH):
            e_h = eatt.tile([128, KB, SL], FP8, name=f"e{h}", tag="eh")
            for k4 in range(KB // 4):
                ps = ps_sc.tile([128, 4 * SL], F32, name="scps", tag="scps")
                for j in range(4):
                    kb = k4 * 4 + j
                    r, lb = divmod(kb, 2)
                    nc.tensor.matmul(ps[:, j * SL:(j + 1) * SL],
                                     lhsT=ktr[r][:, h,
                                                 lb * 128:(lb + 1) * 128],
                                     rhs=qt[h], start=True, stop=True)
                ev = e_h[:, k4 * 4:(k4 + 1) * 4, :].rearrange("p a b -> p (a b)")
                nc.scalar.activation(out=ev, in_=ps, func=AF.Exp, scale=SCALE)
            pa = ps_at.tile([HD + 1, SL], F32, name="atps", tag="atps")
            for kb in range(KB):
                nc.tensor.matmul(pa,
                                 lhsT=vaug[kb][:, h, :],
                                 rhs=e_h[:, kb, :],
                                 start=(kb == 0), stop=(kb == KB - 1))
            # normalizer: row HD of pa holds Z[q]; move it to partition 0
            # with a tiny DMA, then broadcast 1/Z across partitions via a
            # rank-1 outer-product matmul on the PE.
            ztmp = misc.tile([HD + 1, SL], F32, name="ztmp", tag="ztmp")
            nc.vector.tensor_copy(out=ztmp[64:HD + 1, :], in_=pa[64:HD + 1, :])
            zrow = misc.tile([1, SL], F32, name="zrow", tag="zrow")
            nc.scalar.dma_start(out=zrow, in_=ztmp[HD:HD + 1, :])
            nc.vector.reciprocal_approx_fast(out=zrow, in_=zrow)
            rb = ps_rb.tile([128, SL], F32, name="rbps", tag="rbps")
            nc.tensor.matmul(rb, lhsT=ones_f, rhs=zrow, start=True, stop=True)
            rbs = misc.tile([HD, SL], F32, name="rbs", tag="rbs")
            nc.vector.tensor_copy(out=rbs, in_=rb[0:HD, :])
            nc.vector.tensor_mul(out=attnT[h], in0=pa[0:HD, :], in1=rbs)

        ps_rb.release()
        ps_at.release()
        ps_sc.release()
        eatt.release()
        katt.release()
        p_att.release()

        # ================= phase C: O projection + residual, LN2 =======
        psC_tr = tc.alloc_tile_pool(name="psC_tr", bufs=2, space="PSUM")
        psC_mm = tc.alloc_tile_pool(name="psC_mm", bufs=2, space="PSUM")

        for sb in range(SB):
            for (c0, cn) in NCOLS:
                ps = psC_mm.tile([128, 512], F32, name="mmps", tag="mmps")
                for h in range(NH):
                    nc.tensor.matmul(ps[:, 0:cn],
                                     lhsT=attnT[h][:, sb * 128:(sb + 1) * 128],
                                     rhs=wo_t[h][:, c0:c0 + cn],
                                     start=(h == 0),
                                     stop=(not use_bias and h == NH - 1))
                if use_bias:
                    nc.tensor.matmul(ps[:, 0:cn], lhsT=ones_b,
                                     rhs=bias_t[3][:, c0:c0 + cn],
                                     start=False, stop=True)
                nc.vector.tensor_add(out=x2_sb[sb][:, c0:c0 + cn],
                                     in0=ps[:, 0:cn],
                                     in1=x_sb[sb][:, c0:c0 + cn])

        xln2 = [p_ln2.tile([128, H], BF16, name=f"xln2{sb}", tag=f"xln2{sb}")
                for sb in range(SB)]
        layernorm_bf16(x2_sb, xln2)
        xln2T = [p_ln2.tile([128, SL], BF16, name=f"xln2T{hc}", tag=f"xln2T{hc}")
                 for hc in range(HC)]
        transpose_to(xln2, xln2T, psC_tr)

        psC_mm.release()
        psC_tr.release()
        wop.release()

        # ================= phase D: MLP ================================
        w1p = tc.alloc_tile_pool(name="w1p", bufs=4)
        w2p = tc.alloc_tile_pool(name="w2p", bufs=6)
        gtp = tc.alloc_tile_pool(name="gtp", bufs=1)
        ps_fc1 = tc.alloc_tile_pool(name="ps_fc1", bufs=2, space="PSUM")
        ps_fc2 = tc.alloc_tile_pool(name="ps_fc2", bufs=1, space="PSUM")

        fc2ps = {}
        for sb in range(SB):
            for (c0, cn) in NCOLS:
                fc2ps[(sb, c0)] = ps_fc2.tile([128, 512], F32,
                                              name=f"fc2ps{sb}_{c0}",
                                              tag=f"fc2ps{sb}_{c0}")
        w2q = [nc.sync, nc.scalar]
        for mb in range(MC):
            if mb < W1PRE:
                w1 = w1pre_t[mb]
            else:
                w1 = w1p.tile([128, HC, 128], BF16, name=f"w1_{mb}", tag="w1")
                nc.gpsimd.dma_start(out=w1, in_=w1b_io[mb])
            p1 = ps_fc1.tile([128, SL], F32, name="fc1ps", tag="fc1ps")
            for hc in range(HC):
                nc.tensor.matmul(p1, lhsT=w1[:, hc, :], rhs=xln2T[hc],
                                 start=(hc == 0), stop=(hc == HC - 1))
            gt = gtp.tile([128, SL], BF16, name=f"gt{mb}", tag=f"gt{mb}")
            nc.scalar.activation(out=gt, in_=p1, func=AF.Silu,
                                 scale=1.702, bias=b1s[:, mb:mb + 1])
            w2 = w2p.tile([128, H], BF16, name=f"w2_{mb}", tag="w2")
            w2q[mb % 2].dma_start(out=w2, in_=w2t_io[mb * 128:(mb + 1) * 128, :])
            for sb in range(SB):
                for (c0, cn) in NCOLS:
                    nc.tensor.matmul(fc2ps[(sb, c0)][:, 0:cn],
                                     lhsT=gt[:, sb * 128:(sb + 1) * 128],
                                     rhs=w2[:, c0:c0 + cn],
                                     start=(mb == 0),
                                     stop=(not use_bias and mb == MC - 1))
        outsb = [persist.tile([128, H], F32, name=f"o{sb}", tag=f"o{sb}")
                 for sb in range(SB)]
        for sb in range(SB):
            for (c0, cn) in NCOLS:
                if use_bias:
                    nc.tensor.matmul(fc2ps[(sb, c0)][:, 0:cn], lhsT=ones_b,
                                     rhs=bias_t[4][:, c0:c0 + cn],
                                     start=False, stop=True)
                nc.vector.tensor_add(out=outsb[sb][:, c0:c0 + cn],
                                     in0=fc2ps[(sb, c0)][:, 0:cn],
                                     in1=x2_sb[sb][:, c0:c0 + cn])
                w2q[(sb + c0) % 2].dma_start(
                    out=out_io[sb * 128:(sb + 1) * 128, c0:c0 + cn],
                    in_=outsb[sb][:, c0:c0 + cn])

        ps_fc2.release()
        ps_fc1.release()
        gtp.release()
        w2p.release()
        w1p.release()
        p_ln2.release()
        w1pre.release()
        misc.release()
        persist.release()
        const.release()

    nc.compile()
    return nc


_NC = {}


def _get_nc(use_bias=False):
    if use_bias not in _NC:
        _NC[use_bias] = _build_bass(use_bias)
    return _NC[use_bias]


def _prep_inputs(hidden_states, cos, sin,
                 ln1_g, ln1_b, ln2_g, ln2_b,
                 Wq, bq, Wk, bk, Wv, bv, Wo, bo,
                 W1, b1, W2, b2):
    f32 = np.float32
    x = np.asarray(hidden_states, f32).reshape(S, H)
    cos = np.asarray(cos, f32)
    sin = np.asarray(sin, f32)
    g1 = np.asarray(ln1_g, f32); be1 = np.asarray(ln1_b, f32)
    g2 = np.asarray(ln2_g, f32); be2 = np.asarray(ln2_b, f32)
    Wq = np.asarray(Wq, f32); Wk = np.asarray(Wk, f32); Wv = np.asarray(Wv, f32)
    Wo = np.asarray(Wo, f32); W1 = np.asarray(W1, f32); W2 = np.asarray(W2, f32)

    # fold LN1 affine into QKV, LN2 affine into fc1 (exact in fp32).
    # QKV weights ship as fp8 e4m3 scaled x64 (values ~0.02 would hit the
    # subnormal range otherwise); the 1/64 is folded into the LN rstd.
    F8 = ml_dtypes.float8_e4m3fn
    wqt = (64.0 * g1[:, None] * Wq.T).astype(F8)
    wkt = (64.0 * g1[:, None] * Wk.T).astype(F8)
    wvt = (64.0 * g1[:, None] * Wv.T).astype(F8)
    bq_e = np.asarray(bq, f32) + Wq @ be1
    bk_e = np.asarray(bk, f32) + Wk @ be1
    bv_e = np.asarray(bv, f32) + Wv @ be1
    wot = Wo.T.astype(BF)
    w1t = g2[:, None] * W1.T                       # [H, MLP]
    w1b = np.ascontiguousarray(
        w1t.reshape(HC, 128, MC, 128).transpose(2, 1, 0, 3)).astype(BF)
    b1_e = np.asarray(b1, f32) + W1 @ be2
    b1s = np.ascontiguousarray(
        (1.702 * b1_e).reshape(MC, 128).T).astype(f32)  # [128, MC]
    w2t = (W2.T / 1.702).astype(BF)                 # gelu scale folded
    bias5 = np.stack([bq_e, bk_e, bv_e,
                      np.asarray(bo, f32), np.asarray(b2, f32)]).astype(BF)

    wsum3 = np.concatenate([wqt.astype(f32).sum(0), wkt.astype(f32).sum(0),
                            wvt.astype(f32).sum(0)]).reshape(1, 3 * H).astype(BF)
    xT = x.T.astype(F8)                             # [H, S]
    cos_rep = np.tile(cos, (1, NH)).astype(BF)      # [S, H]
    sin_sgn = np.concatenate([-sin[:, :40], sin[:, 40:]], axis=1)
    sin_rep = np.tile(sin_sgn, (1, NH)).astype(BF)  # [S, H]

    shared = {
        "wqt": wqt, "wkt": wkt, "wvt": wvt, "wot": wot,
        "w1b": w1b, "w2t": w2t, "bias5": bias5, "b1s": b1s,
        "wsum3": wsum3,
    }
    in_maps = []
    for c in range(NCORES):
        sl = slice(c * SL, (c + 1) * SL)
        m = dict(shared)
        m["x_loc"] = np.ascontiguousarray(x[sl])
        m["xT"] = np.ascontiguousarray(xT[:, sl])
        m["cosr"] = np.ascontiguousarray(cos_rep[sl])
        m["sins"] = np.ascontiguousarray(sin_rep[sl])
        in_maps.append(m)
    return in_maps


def kernel(hidden_states, attention_mask, cos, sin,
           ln1_g, ln1_b, ln2_g, ln2_b,
           Wq, bq, Wk, bk, Wv, bv, Wo, bo,
           W1, b1, W2, b2):
    # attention_mask is all-True for this problem (spec fill: ones); the
    # dense softmax below assumes it.
    from concourse.bass_utils import run_bass_kernel_spmd

    use_bias = any(
        float(np.abs(np.asarray(b, np.float32)).max()) != 0.0
        for b in (bq, bk, bv, bo, b2))
    nc = _get_nc(use_bias)
    in_maps = _prep_inputs(hidden_states, cos, sin,
                           ln1_g, ln1_b, ln2_g, ln2_b,
                           Wq, bq, Wk, bk, Wv, bv, Wo, bo,
                           W1, b1, W2, b2)
    res = run_bass_kernel_spmd(nc, in_maps, core_ids=list(range(NCORES)))
    out = np.concatenate([res.results[c]["out_loc"] for c in range(NCORES)],
                         axis=0)
    return out.reshape(B, S, H).astype(np.float32)


# revision 26
# speedup vs baseline: 1.1077x; 1.1077x over previous
"""Trainium2 Bass kernel for a Qwen2-VL vision transformer block.

Strategy: 8-way sequence-parallel across NeuronCores. Each core owns a
256-row shard of the 2048-token sequence and the full weights (bf16).
K/V for the full sequence are exchanged with two AllGathers (V first,
then K) so the gathers overlap the Q-side projections; every other
stage is perfectly partitioned. All matmuls run on the PE in bf16 with
fp32 PSUM accumulation; LayerNorm statistics, the softmax normalizer,
and residual adds stay in fp32 (RoPE arithmetic is bf16).

Layout notes:
  - Projections produce activations in natural [seq, feat] layout; the
    PE transpose (identity matmul) produces the [feat, seq] operands
    that later matmuls need as stationary input.
  - Attention computes scores^T [key, query] per head so the softmax
    denominator can be accumulated by an extra all-ones column on the
    stationary V operand, and the per-query normalizer is broadcast
    across partitions with a rank-1 outer-product matmul.
  - LayerNorm affine params and the quick-gelu 1.702 scale are folded
    into the weights on the host, which is exact in fp32.
  - W1 is mostly prefetched into SBUF during the attention phase so the
    MLP runs PE-bound instead of DMA-bound.
"""

import os
import sys

import numpy as np

for _p in ("/opt/trn_rl_repo",):
    if _p not in sys.path:
        sys.path.insert(0, _p)

import ml_dtypes  # noqa: E402


BF = ml_dtypes.bfloat16

B, S, H = 1, 2048, 1280
NH, HD = 16, 80
MLP = 5120
EPS = 1e-6
NCORES = 8
SL = S // NCORES            # 256 sequence rows per core
SB = SL // 128              # 2 partition blocks per core
HC = H // 128               # 10 contraction chunks over H
MC = MLP // 128             # 40 blocks over the MLP dim
KB = S // 128               # 16 key blocks over the full sequence
NCOLS = ((0, 512), (512, 512), (1024, 256))
SCALE = 1.0 / float(np.sqrt(np.float32(HD)))
KT_ELEMS = NH * HD * SL     # 327680, gathered K^T region
VROW = NH * (HD + 1)        # 1296, V row with interleaved ones column
V_ELEMS = SL * VROW         # gathered V region (ones travel with it)
KVE = KT_ELEMS + V_ELEMS
NCOLSV = ((0, 320), (320, 320), (640, 320), (960, 320))
W1PRE = 8                   # W1 blocks prefetched into SBUF before MLP


def _build_bass(use_bias):
    import bass_rust
    import concourse.bacc as bacc
    import concourse.tile as tile
    from concourse import mybir
    from concourse.masks import make_identity

    F32 = mybir.dt.float32
    BF16 = mybir.dt.bfloat16
    FP8 = mybir.dt.float8e4
    AF = mybir.ActivationFunctionType
    OP = mybir.AluOpType

    nc = bacc.Bacc("TRN2", target_bir_lowering=False, debug=False,
                   num_devices=NCORES)

    x_io = nc.dram_tensor("x_loc", [SL, H], F32, kind="ExternalInput")
    xt_io = nc.dram_tensor("xT", [H, SL], FP8, kind="ExternalInput")
    wsum_io = nc.dram_tensor("wsum3", [1, 3 * H], BF16, kind="ExternalInput")
    cos_io = nc.dram_tensor("cosr", [SL, H], BF16, kind="ExternalInput")
    sin_io = nc.dram_tensor("sins", [SL, H], BF16, kind="ExternalInput")
    wqt_io = nc.dram_tensor("wqt", [H, H], FP8, kind="ExternalInput")
    wkt_io = nc.dram_tensor("wkt", [H, H], FP8, kind="ExternalInput")
    wvt_io = nc.dram_tensor("wvt", [H, H], FP8, kind="ExternalInput")
    wot_io = nc.dram_tensor("wot", [H, H], BF16, kind="ExternalInput")
    w1b_io = nc.dram_tensor("w1b", [MC, 128, HC, 128], BF16,
                            kind="ExternalInput")
    w2t_io = nc.dram_tensor("w2t", [MLP, H], BF16, kind="ExternalInput")
    bias5_io = nc.dram_tensor("bias5", [5, H], BF16, kind="ExternalInput")
    b1s_io = nc.dram_tensor("b1s", [128, MC], F32, kind="ExternalInput")
    out_io = nc.dram_tensor("out_loc", [SL, H], F32, kind="ExternalOutput")

    cc_in = nc.dram_tensor("cc_in", [KVE], FP8)
    cc_out = nc.dram_tensor("cc_out", [NCORES, KVE], FP8,
                            addr_space="Shared")

    with tile.TileContext(nc) as tc:
        _qrr = [nc.sync, nc.gpsimd, nc.scalar]
        _qi = [0]

        def dmaq():
            e = _qrr[_qi[0] % len(_qrr)]
            _qi[0] += 1
            return e

        const = tc.alloc_tile_pool(name="const", bufs=1)
        persist = tc.alloc_tile_pool(name="persist", bufs=1)
        misc = tc.alloc_tile_pool(name="misc", bufs=2)

        ident = const.tile([128, 128], BF16, name="ident", tag="ident")
        make_identity(nc, ident)
        ones_b = const.tile([1, 128], BF16, name="ones_b", tag="ones_b")
        nc.vector.memset(ones_b, 1.0)
        ones_f = const.tile([1, 128], F32, name="ones_f", tag="ones_f")
        nc.vector.memset(ones_f, 1.0)
        eps_t = const.tile([128, 1], F32, name="eps_t", tag="eps_t")
        nc.vector.memset(eps_t, EPS)
        bias_t = []
        if use_bias:
            for bi in range(5):
                bt = const.tile([1, H], BF16, name=f"bias{bi}",
                                tag=f"bias{bi}")
                nc.sync.dma_start(out=bt, in_=bias5_io[bi:bi + 1, :])
                bias_t.append(bt)
        x_sb = [persist.tile([128, H], F32, name=f"x{sb}", tag=f"x{sb}")
                for sb in range(SB)]
        for sb in range(SB):
            nc.sync.dma_start(out=x_sb[sb], in_=x_io[sb * 128:(sb + 1) * 128, :])
        wsum_t = const.tile([1, 3 * H], BF16, name="wsum", tag="wsum")
        nc.scalar.dma_start(out=wsum_t, in_=wsum_io[:, :])
        b1s = const.tile([128, MC], F32, name="b1s", tag="b1s")
        nc.gpsimd.dma_start(out=b1s, in_=b1s_io[:, :])
        x2_sb = [persist.tile([128, H], F32, name=f"x2_{sb}", tag=f"x2_{sb}")
                 for sb in range(SB)]
        qt = [persist.tile([HD, SL], FP8, name=f"qt{h}", tag=f"qt{h}")
              for h in range(NH)]
        attnT = [persist.tile([HD, SL], BF16, name=f"attnT{h}", tag=f"attnT{h}")
                 for h in range(NH)]

        def layernorm_bf16(src, dst):
            # dst[sb] = (src[sb] - mean) * rsqrt(var + eps), cast to bf16
            for sb in range(SB):
                stats = misc.tile([128, 5, 6], F32, name=f"lnst{sb}", tag="lnst")
                sv = src[sb].rearrange("p (g d) -> p g d", d=256)
                for g in range(5):
                    nc.vector.bn_stats(out=stats[:, g, :], in_=sv[:, g, :])
                mv = misc.tile([128, 2], F32, name=f"lnmv{sb}", tag="lnmv")
                nc.vector.bn_aggr(out=mv, in_=stats)
                rstd = misc.tile([128, 1], F32, name=f"lnrs{sb}", tag="lnrs")
                nc.scalar.activation(out=rstd, in_=mv[:, 1:2], func=AF.Sqrt,
                                     bias=eps_t)
                nc.vector.reciprocal(out=rstd, in_=rstd)
                nc.vector.tensor_scalar(out=dst[sb], in0=src[sb],
                                        scalar1=mv[:, 0:1], scalar2=rstd,
                                        op0=OP.subtract, op1=OP.mult)

        def transpose_to(src, dst, ps_pool):
            # src: SB tiles [128, H] bf16 -> dst: HC tiles [128, SL] bf16
            insts = []
            for hc in range(HC):
                for sb in range(SB):
                    pt = ps_pool.tile([128, 128], BF16, name="pt", tag="pt")
                    tr = nc.tensor.transpose(pt,
                                             src[sb][:, hc * 128:(hc + 1) * 128],
                                             ident)
                    insts.append(tr)
                    nc.vector.tensor_copy(
                        out=dst[hc][:, sb * 128:(sb + 1) * 128], in_=pt)
            return insts

        # ================= phase A: LN1, QKV, RoPE, head transposes ====
        p_ln = tc.alloc_tile_pool(name="p_ln", bufs=1)
        p_qkv = tc.alloc_tile_pool(name="p_qkv", bufs=1)
        wpool = tc.alloc_tile_pool(name="wpool", bufs=2)
        psA_tr = tc.alloc_tile_pool(name="psA_tr", bufs=2, space="PSUM")
        psA_mm = tc.alloc_tile_pool(name="psA_mm", bufs=2, space="PSUM")
        ps_warm = tc.alloc_tile_pool(name="ps_warm", bufs=2, space="PSUM")

        # Keep the PE busy (HAM warm) while the first DMAs and LayerNorm
        # run: a burst of small identity matmuls chained into one
        # accumulator so the scheduler cannot float them into later
        # phases (their PSUM writes would clobber live accumulators).
        WARMUP = True
        if WARMUP:
            wps = ps_warm.tile([128, 128], F32, name="warm", tag="warm")
            warm_insts = []
            for wi in range(8):
                mm = nc.tensor.matmul(wps, lhsT=ident, rhs=ident,
                                      start=(wi == 0), stop=(wi == 7))
                warm_insts.append(mm)

        # x^T comes precomputed from the host (bf16); LayerNorm is applied
        # algebraically: q = r * (x @ W - mu * colsum(W)) via one extra
        # rank-1 matmul row per accumulation group and a per-token scale
        # folded into the PSUM-evacuation copy.
        xTall = p_ln.tile([128, HC, SL], FP8, name="xTall", tag="xTall")
        nc.sync.dma_start(out=xTall,
                          in_=xt_io.rearrange("(hc p) s -> p hc s", p=128))
        mun = []
        rstd_sb = []
        for sb in range(SB):
            stats = misc.tile([128, 5, 6], F32, name=f"lnst{sb}", tag="lnst")
            sv = x_sb[sb].rearrange("p (g d) -> p g d", d=256)
            for g in range(5):
                nc.vector.bn_stats(out=stats[:, g, :], in_=sv[:, g, :])
            mv = misc.tile([128, 2], F32, name=f"lnmv{sb}", tag="lnmv")
            nc.vector.bn_aggr(out=mv, in_=stats)
            rstd = p_ln.tile([128, 1], F32, name=f"rstd{sb}", tag=f"rstd{sb}")
            nc.scalar.activation(out=rstd, in_=mv[:, 1:2], func=AF.Sqrt,
                                 bias=eps_t)
            nc.vector.reciprocal(out=rstd, in_=rstd)
            nc.vector.tensor_scalar(out=rstd, in0=rstd, scalar1=1.0 / 64.0,
                                    scalar2=None, op0=OP.mult)
            rstd_sb.append(rstd)
            mneg = p_ln.tile([128, 1], BF16, name=f"mneg{sb}", tag=f"mneg{sb}")
            nc.vector.tensor_scalar(out=mneg, in0=mv[:, 0:1], scalar1=-1.0,
                                    scalar2=None, op0=OP.mult)
            mrow = p_ln.tile([1, 128], BF16, name=f"mrow{sb}", tag=f"mrow{sb}")
            mps = psA_tr.tile([1, 128], BF16, name=f"mps{sb}", tag="mps")
            nc.tensor.transpose(mps, mneg, ident)
            nc.vector.tensor_copy(out=mrow, in_=mps)
            mun.append(mrow)

        def project(w_io, bias_idx, store, cols=NCOLS, wq_eng=None):
            wt = []
            for hc in range(HC):
                w = wpool.tile([128, H], FP8, name=f"w{hc}", tag=f"w{hc}")
                (wq_eng or dmaq()).dma_start(
                    out=w, in_=w_io[hc * 128:(hc + 1) * 128, :])
                wt.append(w)
            for sb in range(SB):
                for (c0, cn) in cols:
                    ps = psA_mm.tile([128, 512], F32, name="mmps", tag="mmps")
                    for hc in range(HC):
                        nc.tensor.matmul(
                            ps[:, 0:cn],
                            lhsT=xTall[:, hc, sb * 128:(sb + 1) * 128],
                            rhs=wt[hc][:, c0:c0 + cn],
                            start=(hc == 0), stop=False)
                    nc.tensor.matmul(
                        ps[:, 0:cn], lhsT=mun[sb],
                        rhs=wsum_t[:, bias_idx * H + c0:bias_idx * H + c0 + cn],
                        start=False, stop=(not use_bias))
                    if use_bias:
                        nc.tensor.matmul(
                            ps[:, 0:cn], lhsT=ones_b,
                            rhs=bias_t[bias_idx][:, c0:c0 + cn],
                            start=False, stop=True)
                    store(sb, c0, cn, ps)

        # ---- K & V projections; publish both, then one AllGather ----
        # V is stored pre-interleaved with its softmax ones-column so the
        # gathered V can be consumed with contiguous DMAs and no memsets.
        vloc = p_qkv.tile([128, SB, VROW], FP8, name="vloc", tag="vloc")
        v4 = vloc.rearrange("p b (h c) -> p b h c", c=HD + 1)
        nc.vector.memset(v4[:, :, :, HD:HD + 1], 1.0)

        def store_v(sb, c0, cn, ps):
            g0 = c0 // HD
            gn = cn // HD
            nc.scalar.activation(
                out=v4[:, sb, g0:g0 + gn, 0:HD],
                in_=ps[:, 0:cn].rearrange("p (h c) -> p h c", c=HD),
                func=AF.Copy, scale=rstd_sb[sb])

        cosr = [p_qkv.tile([128, H], BF16, name=f"cos{sb}", tag=f"cos{sb}")
                for sb in range(SB)]
        sins = [p_qkv.tile([128, H], BF16, name=f"sin{sb}", tag=f"sin{sb}")
                for sb in range(SB)]
        for sb in range(SB):
            nc.scalar.dma_start(out=cosr[sb],
                                in_=cos_io[sb * 128:(sb + 1) * 128, :])
            nc.scalar.dma_start(out=sins[sb],
                                in_=sin_io[sb * 128:(sb + 1) * 128, :])

        def rope(nat, out):
            for sb in range(SB):
                tmp = misc.tile([128, H], BF16, name="ropetmp", tag="ropetmp")
                t3 = tmp.rearrange("p (h c) -> p h c", c=HD)
                q3 = nat[sb].rearrange("p (h c) -> p h c", c=HD)
                s3 = sins[sb].rearrange("p (h c) -> p h c", c=HD)
                nc.vector.tensor_mul(out=t3[:, :, 0:40], in0=q3[:, :, 40:80],
                                     in1=s3[:, :, 0:40])
                nc.vector.tensor_mul(out=t3[:, :, 40:80], in0=q3[:, :, 0:40],
                                     in1=s3[:, :, 40:80])
                nc.vector.tensor_mul(out=nat[sb], in0=nat[sb], in1=cosr[sb])
                nc.vector.tensor_add(out=out[sb], in0=nat[sb], in1=tmp)

        knat = [p_qkv.tile([128, H], BF16, name=f"kn{sb}", tag=f"kn{sb}")
                for sb in range(SB)]
        krope = [p_qkv.tile([128, H], BF16, name=f"kr{sb}", tag=f"kr{sb}")
                 for sb in range(SB)]
        project(wkt_io, 1,
                lambda sb, c0, cn, ps: nc.scalar.activation(
                    out=knat[sb][:, c0:c0 + cn], in_=ps[:, 0:cn],
                    func=AF.Copy, scale=rstd_sb[sb]),
                wq_eng=nc.gpsimd)
        rope(knat, krope)
        project(wvt_io, 2, store_v, cols=NCOLSV, wq_eng=nc.gpsimd)
        nc.sync.dma_start(
            out=cc_in[KT_ELEMS:KVE].rearrange("(b p f) -> p b f",
                                              p=128, f=VROW),
            in_=vloc)
        ktall = p_qkv.tile([HD, NH, SL], FP8, name="ktall", tag="ktall")
        for h in range(NH):
            for sb in range(SB):
                ptk = psA_tr.tile([HD, 128], BF16, name="ptk", tag="pt")
                nc.tensor.transpose(ptk, krope[sb][:, h * HD:(h + 1) * HD], ident)
                nc.vector.tensor_copy(
                    out=ktall[:, h, sb * 128:(sb + 1) * 128], in_=ptk)
        nc.sync.dma_start(
            out=cc_in[0:KT_ELEMS].rearrange("(d h s) -> d h s", h=NH, s=SL),
            in_=ktall)
        bar_cc = nc.gpsimd.collective_compute(
            "AllGather", OP.bypass,
            replica_groups=[list(range(NCORES))],
            ins=[cc_in.ap()], outs=[cc_out.ap()])

        # ---- Q: project, RoPE, per-head transpose (overlaps gathers) ----
        qnat = [p_qkv.tile([128, H], BF16, name=f"qn{sb}", tag=f"qn{sb}")
                for sb in range(SB)]
        qrope = [p_qkv.tile([128, H], BF16, name=f"qr{sb}", tag=f"qr{sb}")
                 for sb in range(SB)]
        project(wqt_io, 0,
                lambda sb, c0, cn, ps: nc.scalar.activation(
                    out=qnat[sb][:, c0:c0 + cn], in_=ps[:, 0:cn],
                    func=AF.Copy, scale=rstd_sb[sb]))
        rope(qnat, qrope)
        for h in range(NH):
            for sb in range(SB):
                ptq = psA_tr.tile([HD, 128], BF16, name="ptq", tag="pt")
                nc.tensor.transpose(ptq, qrope[sb][:, h * HD:(h + 1) * HD], ident)
                nc.vector.tensor_copy(out=qt[h][:, sb * 128:(sb + 1) * 128],
                                      in_=ptq)

        ps_warm.release()
        psA_mm.release()
        psA_tr.release()
        wpool.release()
        p_qkv.release()
        p_ln.release()

        # ================= phase B: attention ==========================
        # W1 prefetch streams on the gpsimd queue while attention runs.
        w1pre = tc.alloc_tile_pool(name="w1pre", bufs=1)
        w1pre_t = []
        w1q = [nc.sync, nc.scalar]
        for mb in range(W1PRE):
            w1 = w1pre.tile([128, HC, 128], BF16, name=f"w1p{mb}",
                            tag=f"w1p{mb}")
            w1q[mb % 2].dma_start(out=w1, in_=w1b_io[mb])
            w1pre_t.append(w1)

        p_ln2 = tc.alloc_tile_pool(name="p_ln2", bufs=1)
        wop = tc.alloc_tile_pool(name="wop", bufs=1)
        wo_t = []
        for h in range(NH):
            w = wop.tile([HD, H], BF16, name=f"wo{h}", tag=f"wo{h}")
            dmaq().dma_start(out=w, in_=wot_io[h * HD:(h + 1) * HD, :])
            wo_t.append(w)

        p_att = tc.alloc_tile_pool(name="p_att", bufs=1)
        katt = tc.alloc_tile_pool(name="katt", bufs=1)
        eatt = tc.alloc_tile_pool(name="eatt", bufs=2)
        ps_sc = tc.alloc_tile_pool(name="ps_sc", bufs=2, space="PSUM")
        ps_at = tc.alloc_tile_pool(name="ps_at", bufs=3, space="PSUM")
        ps_rb = tc.alloc_tile_pool(name="ps_rb", bufs=1, space="PSUM")

        vaug = []
        for kb in range(KB):
            va = p_att.tile([128, NH, HD + 1], FP8, name=f"vaug{kb}",
                            tag=f"vaug{kb}")
            r, lb = divmod(kb, 2)
            vsrc = cc_out[r, KT_ELEMS + lb * 128 * VROW:
                          KT_ELEMS + (lb + 1) * 128 * VROW]
            vsrc = vsrc.rearrange("(p f) -> p f", f=VROW)
            vdma = nc.scalar.dma_start(
                out=va.rearrange("p a b -> p (a b)"), in_=vsrc)
            bass_rust.add_dep_helper(vdma.ins, bar_cc.ins,
                                     reason="wait for remote V via barrier")
            vaug.append(va)

        ktr = []
        ktq = [nc.sync, nc.scalar]
        for r in range(NCORES):
            krt = katt.tile([HD, NH, SL], FP8, name=f"ktr{r}", tag=f"ktr{r}")
            src_r = cc_out[r, 0:KT_ELEMS].rearrange("(d h s) -> d h s",
                                                    h=NH, s=SL)
            kdma = ktq[r % 2].dma_start(out=krt, in_=src_r)
            bass_rust.add_dep_helper(kdma.ins, bar_cc.ins,
                                     reason="wait for remote K via barrier")
            ktr.append(krt)

        for h in range(NH):
            e_h = eatt.tile([128, KB, SL], FP8, name=f"e{h}", tag="eh")
            for k4 in range(KB // 4):
                ps = ps_sc.tile([128, 4 * SL], F32, name="scps", tag="scps")
                for j in range(4):
                    kb = k4 * 4 + j
                    r, lb = divmod(kb, 2)
                    nc.tensor.matmul(ps[:, j * SL:(j + 1) * SL],
                                     lhsT=ktr[r][:, h,
                                                 lb * 128:(lb + 1) * 128],
                                     rhs=qt[h], start=True, stop=True)
                ev = e_h[:, k4 * 4:(k4 + 1) * 4, :].rearrange("p a b -> p (a b)")
                nc.scalar.activation(out=ev, in_=ps, func=AF.Exp, scale=SCALE)
            pa = ps_at.tile([HD + 1, SL], F32, name="atps", tag="atps")
            for kb in range(KB):
                nc.tensor.matmul(pa,
                                 lhsT=vaug[kb][:, h, :],
                                 rhs=e_h[:, kb, :],
                                 start=(kb == 0), stop=(kb == KB - 1))
            # normalizer: row HD of pa holds Z[q]; move it to partition 0
            # with a tiny DMA, then broadcast 1/Z across partitions via a
            # rank-1 outer-product matmul on the PE.
            ztmp = misc.tile([HD + 1, SL], F32, name="ztmp", tag="ztmp")
            nc.vector.tensor_copy(out=ztmp[64:HD + 1, :], in_=pa[64:HD + 1, :])
            zrow = misc.tile([1, SL], F32, name="zrow", tag="zrow")
            nc.scalar.dma_start(out=zrow, in_=ztmp[HD:HD + 1, :])
            nc.vector.reciprocal_approx_fast(out=zrow, in_=zrow)
            rb = ps_rb.tile([128, SL], F32, name="rbps", tag="rbps")
            nc.tensor.matmul(rb, lhsT=ones_f, rhs=zrow, start=True, stop=True)
            rbs = misc.tile([HD, SL], F32, name="rbs", tag="rbs")
            nc.vector.tensor_copy(out=rbs, in_=rb[0:HD, :])
            nc.vector.tensor_mul(out=attnT[h], in0=pa[0:HD, :], in1=rbs)

        ps_rb.release()
        ps_at.release()
        ps_sc.release()
        eatt.release()
        katt.release()
        p_att.release()

        # ================= phase C: O projection + residual, LN2 =======
        psC_tr = tc.alloc_tile_pool(name="psC_tr", bufs=2, space="PSUM")
        psC_mm = tc.alloc_tile_pool(name="psC_mm", bufs=2, space="PSUM")

        for sb in range(SB):
            for (c0, cn) in NCOLS:
                ps = psC_mm.tile([128, 512], F32, name="mmps", tag="mmps")
                for h in range(NH):
                    nc.tensor.matmul(ps[:, 0:cn],
                                     lhsT=attnT[h][:, sb * 128:(sb + 1) * 128],
                                     rhs=wo_t[h][:, c0:c0 + cn],
                                     start=(h == 0),
                                     stop=(not use_bias and h == NH - 1))
                if use_bias:
                    nc.tensor.matmul(ps[:, 0:cn], lhsT=ones_b,
                                     rhs=bias_t[3][:, c0:c0 + cn],
                                     start=False, stop=True)
                nc.vector.tensor_add(out=x2_sb[sb][:, c0:c0 + cn],
                                     in0=ps[:, 0:cn],
                                     in1=x_sb[sb][:, c0:c0 + cn])

        xln2 = [p_ln2.tile([128, H], BF16, name=f"xln2{sb}", tag=f"xln2{sb}")
                for sb in range(SB)]
        layernorm_bf16(x2_sb, xln2)
        xln2T = [p_ln2.tile([128, SL], BF16, name=f"xln2T{hc}", tag=f"xln2T{hc}")
                 for hc in range(HC)]
        transpose_to(xln2, xln2T, psC_tr)

        psC_mm.release()
        psC_tr.release()
        wop.release()

        # ================= phase D: MLP ================================
        w1p = tc.alloc_tile_pool(name="w1p", bufs=4)
        w2p = tc.alloc_tile_pool(name="w2p", bufs=6)
        gtp = tc.alloc_tile_pool(name="gtp", bufs=1)
        ps_fc1 = tc.alloc_tile_pool(name="ps_fc1", bufs=2, space="PSUM")
        ps_fc2 = tc.alloc_tile_pool(name="ps_fc2", bufs=1, space="PSUM")

        fc2ps = {}
        for sb in range(SB):
            for (c0, cn) in NCOLS:
                fc2ps[(sb, c0)] = ps_fc2.tile([128, 512], F32,
                                              name=f"fc2ps{sb}_{c0}",
                                              tag=f"fc2ps{sb}_{c0}")
        w2q = [nc.sync, nc.scalar]
        for mb in range(MC):
            if mb < W1PRE:
                w1 = w1pre_t[mb]
            else:
                w1 = w1p.tile([128, HC, 128], BF16, name=f"w1_{mb}", tag="w1")
                nc.gpsimd.dma_start(out=w1, in_=w1b_io[mb])
            p1 = ps_fc1.tile([128, SL], F32, name="fc1ps", tag="fc1ps")
            for hc in range(HC):
                nc.tensor.matmul(p1, lhsT=w1[:, hc, :], rhs=xln2T[hc],
                                 start=(hc == 0), stop=(hc == HC - 1))
            gt = gtp.tile([128, SL], BF16, name=f"gt{mb}", tag=f"gt{mb}")
            nc.scalar.activation(out=gt, in_=p1, func=AF.Silu,
                                 scale=1.702, bias=b1s[:, mb:mb + 1])
            w2 = w2p.tile([128, H], BF16, name=f"w2_{mb}", tag="w2")
            w2q[mb % 2].dma_start(out=w2, in_=w2t_io[mb * 128:(mb + 1) * 128, :])
            for sb in range(SB):
                for (c0, cn) in NCOLS:
                    nc.tensor.matmul(fc2ps[(sb, c0)][:, 0:cn],
                                     lhsT=gt[:, sb * 128:(sb + 1) * 128],
                                     rhs=w2[:, c0:c0 + cn],
                                     start=(mb == 0),
                                     stop=(not use_bias and mb == MC - 1))
        outsb = [persist.tile([128, H], F32, name=f"o{sb}", tag=f"o{sb}")
                 for sb in range(SB)]
        for sb in range(SB):
            for (c0, cn) in NCOLS:
                if use_bias:
                    nc.tensor.matmul(fc2ps[(sb, c0)][:, 0:cn], lhsT=ones_b,
                                     rhs=bias_t[4][:, c0:c0 + cn],
                                     start=False, stop=True)
                nc.vector.tensor_add(out=outsb[sb][:, c0:c0 + cn],
                                     in0=fc2ps[(sb, c0)][:, 0:cn],
                                     in1=x2_sb[sb][:, c0:c0 + cn])
                w2q[(sb + c0) % 2].dma_start(
                    out=out_io[sb * 128:(sb + 1) * 128, c0:c0 + cn],
                    in_=outsb[sb][:, c0:c0 + cn])

        ps_fc2.release()
        ps_fc1.release()
        gtp.release()
        w2p.release()
        w1p.release()
        p_ln2.release()
        w1pre.release()
        misc.release()
        persist.release()
        const.release()

    nc.compile()
    return nc


_NC = {}


def _get_nc(use_bias=False):
    if use_bias not in _NC:
        _NC[use_bias] = _build_bass(use_bias)
    return _NC[use_bias]


def _prep_inputs(hidden_states, cos, sin,
                 ln1_g, ln1_b, ln2_g, ln2_b,
                 Wq, bq, Wk, bk, Wv, bv, Wo, bo,
                 W1, b1, W2, b2):
    f32 = np.float32
    x = np.asarray(hidden_states, f32).reshape(S, H)
    cos = np.asarray(cos, f32)
    sin = np.asarray(sin, f32)
    g1 = np.asarray(ln1_g, f32); be1 = np.asarray(ln1_b, f32)
    g2 = np.asarray(ln2_g, f32); be2 = np.asarray(ln2_b, f32)
    Wq = np.asarray(Wq, f32); Wk = np.asarray(Wk, f32); Wv = np.asarray(Wv, f32)
    Wo = np.asarray(Wo, f32); W1 = np.asarray(W1, f32); W2 = np.asarray(W2, f32)

    # fold LN1 affine into QKV, LN2 affine into fc1 (exact in fp32).
    # QKV weights ship as fp8 e4m3 scaled x64 (values ~0.02 would hit the
    # subnormal range otherwise); the 1/64 is folded into the LN rstd.
    F8 = ml_dtypes.float8_e4m3fn
    wqt = (64.0 * g1[:, None] * Wq.T).astype(F8)
    wkt = (64.0 * g1[:, None] * Wk.T).astype(F8)
    wvt = (64.0 * g1[:, None] * Wv.T).astype(F8)
    bq_e = np.asarray(bq, f32) + Wq @ be1
    bk_e = np.asarray(bk, f32) + Wk @ be1
    bv_e = np.asarray(bv, f32) + Wv @ be1
    wot = Wo.T.astype(BF)
    w1t = g2[:, None] * W1.T                       # [H, MLP]
    w1b = np.ascontiguousarray(
        w1t.reshape(HC, 128, MC, 128).transpose(2, 1, 0, 3)).astype(BF)
    b1_e = np.asarray(b1, f32) + W1 @ be2
    b1s = np.ascontiguousarray(
        (1.702 * b1_e).reshape(MC, 128).T).astype(f32)  # [128, MC]
    w2t = (W2.T / 1.702).astype(BF)                 # gelu scale folded
    bias5 = np.stack([bq_e, bk_e, bv_e,
                      np.asarray(bo, f32), np.asarray(b2, f32)]).astype(BF)

    wsum3 = np.concatenate([wqt.astype(f32).sum(0), wkt.astype(f32).sum(0),
                            wvt.astype(f32).sum(0)]).reshape(1, 3 * H).astype(BF)
    xT = x.T.astype(F8)                             # [H, S]
    cos_rep = np.tile(cos, (1, NH)).astype(BF)      # [S, H]
    sin_sgn = np.concatenate([-sin[:, :40], sin[:, 40:]], axis=1)
    sin_rep = np.tile(sin_sgn, (1, NH)).astype(BF)  # [S, H]

    shared = {
        "wqt": wqt, "wkt": wkt, "wvt": wvt, "wot": wot,
        "w1b": w1b, "w2t": w2t, "bias5": bias5, "b1s": b1s,
        "wsum3": wsum3,
    }
    in_maps = []
    for c in range(NCORES):
        sl = slice(c * SL, (c + 1) * SL)
        m = dict(shared)
        m["x_loc"] = np.ascontiguousarray(x[sl])
        m["xT"] = np.ascontiguousarray(xT[:, sl])
        m["cosr"] = np.ascontiguousarray(cos_rep[sl])
        m["sins"] = np.ascontiguousarray(sin_rep[sl])
        in_maps.append(m)
    return in_maps


def kernel(hidden_states, attention_mask, cos, sin,
           ln1_g, ln1_b, ln2_g, ln2_b,
           Wq, bq, Wk, bk, Wv, bv, Wo, bo,
           W1, b1, W2, b2):
    # attention_mask is all-True for this problem (spec fill: ones); the
    # dense softmax below assumes it.
    from concourse.bass_utils import run_bass_kernel_spmd

    use_bias = any(
        float(np.abs(np.asarray(b, np.float32)).max()) != 0.0
        for b in (bq, bk, bv, bo, b2))
    nc = _get_nc(use_bias)
    in_maps = _prep_inputs(hidden_states, cos, sin,
                           ln1_g, ln1_b, ln2_g, ln2_b,
                           Wq, bq, Wk, bk, Wv, bv, Wo, bo,
                           W1, b1, W2, b2)
    res = run_bass_kernel_spmd(nc, in_maps, core_ids=list(range(NCORES)))
    out = np.concatenate([res.results[c]["out_loc"] for c in range(NCORES)],
                         axis=0)
    return out.reshape(B, S, H).astype(np.float32)
